# revision 25
# baseline (speedup 1.0000x reference)
"""MoE feed-forward (top-2 of 8 experts) on 8 Trainium2 NeuronCores.

Sharding: expert-parallel (1 expert per core), router replicated
data-parallel over the token dim (each core routes its 512-token shard,
then an AllGather shares routing results).

Per-core pipeline:
  1. Router: PE-transpose own token shard (fp32), fp32 matmul vs router_w,
     softmax/top-2 on DVE/ACT, AllGather of (e1, e2, w1, w2) + aux partials.
  2. Compaction: build "token id if my expert else -1" in wrap-16 layout,
     gpsimd sparse_gather -> compact token list + gate list. Pad to a fixed
     capacity C with trash-token 4096 so everything downstream is static.
  3. dma_gather token rows (fp32), cast bf16, PE-transpose -> xT tiles.
  4. FFN: h^T = gelu(W1^T x^T + b1) (bf16 matmuls, fp32 psum, exact-erf
     gelu on ACT), y = h W2 + b2, gate-scale, dma_scatter_add into a dense
     [4096, 1024] fp32 accumulator in HBM.
  5. ReduceScatter(add) across the 8 cores -> each core owns the final
     output rows for its 512-token shard. aux_loss from AllGathered stats.
"""

import os
import sys

for _p in ("/opt/trn_rl_repo",):
    if _p not in sys.path and os.path.isdir(_p):
        sys.path.insert(0, _p)

import numpy as np
import ml_dtypes

import concourse.bass as bass
import concourse.bacc as bacc
import concourse.mybir as mybir
import concourse.tile as tile
from concourse.bass_utils import run_bass_kernel_spmd

dt = mybir.dt
AF = mybir.ActivationFunctionType
ALU = mybir.AluOpType
ts = bass.ts

NCORES = 8
T = 4096          # total tokens
TS = T // NCORES  # tokens per shard (512)
D = 1024          # d_model
F = 2048          # d_ff
E = 8             # experts
C = 1280          # per-expert token capacity (observed max load ~1091)
CW = C // 16      # wrap-16 columns of the capacity (80)
TRASH = T         # trash token id for padded slots (row T of padded x)
XPAD = T + 8      # padded x rows
# slot chunks for gather/FFN/scatter pipelining
CHUNKS = [(0, 512), (512, 512), (1024, 256)]


def build_nc(phases=4, skip=()):
    # phases: 1=router+AG only, 2=+compaction, 3=+gather/FFN/scatter, 4=+RS
    # skip: subset of {'ag','sg','gather','scatter'} replaced by stubs
    nc = bacc.Bacc("TRN2", target_bir_lowering=False, debug=False,
                   num_devices=NCORES, num_swdge_queues=1)

    # ---- I/O ----
    xs = nc.dram_tensor("xs", [TS, D], dt.float32, kind="ExternalInput")
    xp = nc.dram_tensor("xp", [XPAD, D], dt.float32, kind="ExternalInput")
    rw = nc.dram_tensor("rw", [D, E], dt.float32, kind="ExternalInput")
    rbT = nc.dram_tensor("rbT", [E, 1], dt.float32, kind="ExternalInput")
    w1 = nc.dram_tensor("w1", [D, F], dt.float32, kind="ExternalInput")
    b1T = nc.dram_tensor("b1T", [128, F // 128], dt.float32, kind="ExternalInput")
    w2 = nc.dram_tensor("w2", [F, D], dt.float32, kind="ExternalInput")
    b2T = nc.dram_tensor("b2T", [128, D], dt.float32, kind="ExternalInput")
    identf = nc.dram_tensor("identf", [128, 128], dt.float32, kind="ExternalInput")
    identb = nc.dram_tensor("identb", [128, 128], dt.bfloat16, kind="ExternalInput")
    iotaE = nc.dram_tensor("iotaE", [128, E], dt.float32, kind="ExternalInput")
    onesP = nc.dram_tensor("onesP", [128, 1], dt.float32, kind="ExternalInput")
    iwrap1 = nc.dram_tensor("iwrap1", [16, T // 16], dt.float32, kind="ExternalInput")
    iwrapC = nc.dram_tensor("iwrapC", [16, CW], dt.float32, kind="ExternalInput")
    myexp = nc.dram_tensor("myexp", [16, 1], dt.float32, kind="ExternalInput")

    out = nc.dram_tensor("out", [TS, D], dt.float32, kind="ExternalOutput")
    aux = nc.dram_tensor("aux", [1, 1], dt.float32, kind="ExternalOutput")

    # ---- internal DRAM ----
    agin1 = nc.dram_tensor("agin1", [TS * 4], dt.float32)
    agout1 = nc.dram_tensor("agout1", [NCORES * TS * 4], dt.float32,
                            addr_space="Shared")
    agin2 = nc.dram_tensor("agin2", [64], dt.float32)
    agout2 = nc.dram_tensor("agout2", [NCORES * 64], dt.float32,
                            addr_space="Shared")
    idxdram = nc.dram_tensor("idxdram", [C], dt.int16)
    nfdram = nc.dram_tensor("nfdram", [16], dt.float32)
    gdram = nc.dram_tensor("gdram", [C], dt.float32)
    acc = nc.dram_tensor("acc", [T + 1, D], dt.float32)
    rsout = nc.dram_tensor("rsout", [TS, D], dt.float32)

    rg = [list(range(NCORES))]

    with tile.TileContext(nc) as tc, tc.tile_pool(name="persist", bufs=1) as pp:
        with (
            tc.tile_pool(name="wstage", bufs=1) as wsp,
            tc.tile_pool(name="early", bufs=1) as ep,
        ):
            # ---------- constants into SBUF ----------
            identf_sb = pp.tile([128, 128], dt.float32, tag="identf")
            nc.sync.dma_start(identf_sb[:], identf[:, :])
            identb_sb = pp.tile([128, 128], dt.bfloat16, tag="identb")
            nc.sync.dma_start(identb_sb[:], identb[:, :])
            b1_sb = pp.tile([128, F // 128], dt.float32, tag="b1")
            nc.sync.dma_start(b1_sb[:], b1T[:, :])
            b2_sb = pp.tile([128, D], dt.float32, tag="b2")
            nc.sync.dma_start(b2_sb[:], b2T[:, :])

            # ---------- zero the accumulator ----------
            zer = ep.tile([128, D], dt.float32, tag="zeros")
            nc.vector.memset(zer[:], 0.0)
            for z in range(T // 128):
                nc.sync.dma_start(acc[ts(z, 128), :], zer[:])

            # ---------- expert weights: load fp32, cast bf16 ----------
            # staging tiles get slice-writes (no waits); wave-2 tiles are
            # pre-touched with an ACT memzero to absorb slot-release deps
            w1_sb = pp.tile([128, D // 128, F], dt.bfloat16, tag="w1bf")
            w2_sb = pp.tile([128, F // 128, D], dt.bfloat16, tag="w2bf")
            for wave in range(2):
                stg = wsp.tile([128, 4, F], dt.float32, tag="w1stage")
                if wave:
                    nc.scalar.memzero(stg[:])
                for j in range(4):
                    k = wave * 4 + j
                    nc.sync.dma_start(stg[:, j, :], w1[ts(k, 128), :])
                    nc.scalar.copy(w1_sb[:, k, :], stg[:, j, :])
            for wave in range(2):
                stg = wsp.tile([128, 8, D], dt.float32, tag="w2stage")
                if wave:
                    nc.scalar.memzero(stg[:])
                for j in range(8):
                    k = wave * 8 + j
                    nc.sync.dma_start(stg[:, j, :], w2[ts(k, 128), :])
                    nc.vector.tensor_copy(w2_sb[:, k, :], stg[:, j, :])

        # =========== PHASE 1: router on my 512-token shard ===========
        with (
            tc.tile_pool(name="router", bufs=1) as rp,
            tc.tile_pool(name="rpsum", bufs=2, space="PSUM") as rps,
        ):
            rw_sb = rp.tile([128, D // 128, E], dt.float32, tag="rw")
            nc.scalar.memzero(rw_sb[:])
            nc.sync.dma_start(rw_sb[:], rw[:, :].rearrange("(k p) e -> p k e", p=128))
            rb_sb = rp.tile([E, 1], dt.float32, tag="rb")
            nc.scalar.memzero(rb_sb[:])
            nc.sync.dma_start(rb_sb[:], rbT[:, :])
            iotaE_sb = rp.tile([128, E], dt.float32, tag="iotaE")
            nc.scalar.memzero(iotaE_sb[:])
            nc.sync.dma_start(iotaE_sb[:], iotaE[:, :])
            ones_sb = rp.tile([128, 1], dt.float32, tag="ones")
            nc.scalar.memzero(ones_sb[:])
            nc.sync.dma_start(ones_sb[:], onesP[:, :])

            # transpose shard: xT[p, k, 128g+q] = xs[128g+q, 128k+p]
            xT = rp.tile([128, D // 128, TS], dt.float32, tag="xT")
            xs_sb = rp.tile([128, TS // 128, D], dt.float32, tag="xs_sb")
            nc.vector.tensor_copy(xs_sb[:], xs_sb[:])
            for g in range(TS // 128):
                nc.sync.dma_start(xs_sb[:, g, :], xs[ts(g, 128), :])
                for k in range(D // 128):
                    ptp = rps.tile([128, 128], dt.float32, tag="ptp")
                    nc.tensor.transpose(ptp[:], xs_sb[:, g, ts(k, 128)],
                                        identf_sb[:])
                    nc.vector.tensor_copy(xT[:, k, ts(g, 128)], ptp[:])

            # logitsT[e, t] = sum_k rw[k*128+p, e] * xT[p, k, t]  (fp32)
            p_r = rps.tile([E, TS], dt.float32, tag="p_r")
            for k in range(D // 128):
                nc.tensor.matmul(p_r[:], rw_sb[:, k, :], xT[:, k, :],
                                 start=(k == 0), stop=(k == D // 128 - 1))
            logT = rp.tile([E, TS], dt.float32, tag="logT")
            # add router bias (per-partition scalar since partitions = experts)
            nc.scalar.activation(logT[:], p_r[:], AF.Identity, bias=rb_sb[:, 0:1])

            # transpose back to [tok, e] in 4 groups of 128 tokens
            logits = rp.tile([128, TS // 128, E], dt.float32, tag="logits")
            for g in range(TS // 128):
                ptp2 = rps.tile([128, E], dt.float32, tag="ptp2")
                nc.tensor.transpose(ptp2[:], logT[:, ts(g, 128)], identf_sb[0:8, 0:8])
                nc.vector.tensor_copy(logits[:, g, :], ptp2[:])

            NG = TS // 128  # 4 token groups
            # softmax over E (free dim)
            mx = rp.tile([128, NG], dt.float32, tag="mx")
            nc.vector.tensor_reduce(mx[:], logits[:], mybir.AxisListType.X, ALU.max)
            nmx = rp.tile([128, NG], dt.float32, tag="nmx")
            nc.vector.tensor_scalar_mul(nmx[:], mx[:], -1.0)
            probs = rp.tile([128, NG, E], dt.float32, tag="probs")
            for g in range(NG):
                nc.scalar.activation(probs[:, g, :], logits[:, g, :], AF.Exp,
                                     bias=nmx[:, g:g + 1])
            sm = rp.tile([128, NG], dt.float32, tag="sm")
            nc.vector.tensor_reduce(sm[:], probs[:], mybir.AxisListType.X, ALU.add)
            rsm = rp.tile([128, NG], dt.float32, tag="rsm")
            nc.vector.reciprocal(rsm[:], sm[:])
            for g in range(NG):
                nc.vector.tensor_scalar_mul(probs[:, g, :], probs[:, g, :],
                                            rsm[:, g:g + 1])

            # top-2 selection on logits (order-equivalent to probs)
            maxv = rp.tile([128, NG * 8], dt.float32, tag="maxv")
            maxi = rp.tile([128, NG * 8], dt.uint32, tag="maxi")
            for g in range(NG):
                nc.vector.max(maxv[:, ts(g, 8)], logits[:, g, :])
                nc.vector.max_index(maxi[:, ts(g, 8)], maxv[:, ts(g, 8)],
                                    logits[:, g, :])
            maxv3 = maxv[:].rearrange("p (g e) -> p g e", g=NG)
            maxi3 = maxi[:].rearrange("p (g e) -> p g e", g=NG)

            # fields per token: [e1, e2, w1, w2]
            fields = rp.tile([128, NG, 4], dt.float32, tag="fields")
            nc.vector.tensor_copy(fields[:, :, 0:1], maxi3[:, :, 0:1])
            nc.vector.tensor_copy(fields[:, :, 1:2], maxi3[:, :, 1:2])
            ldiff = rp.tile([128, NG, 1], dt.float32, tag="ldiff")
            nc.vector.tensor_tensor(ldiff[:], maxv3[:, :, 0:1], maxv3[:, :, 1:2],
                                    ALU.subtract)
            w1t = rp.tile([128, NG, 1], dt.float32, tag="w1t")
            for g in range(NG):
                nc.scalar.activation(w1t[:, g, :], ldiff[:, g, :], AF.Sigmoid)
            nc.vector.tensor_copy(fields[:, :, 2:3], w1t[:])
            nc.vector.tensor_scalar(fields[:, :, 3:4], w1t[:],
                                    -1.0, 1.0, ALU.mult, ALU.add)

            # aux-loss partials: per (group, e) sums of probs and onehot counts
            statpk = rp.tile([128, 2 * NG * E], dt.float32, tag="statpk")
            nc.vector.tensor_copy(statpk[:, 0:NG * E], probs[:])
            oh = rp.tile([128, NG, E], dt.float32, tag="oh")
            st2 = statpk[:, NG * E:2 * NG * E].rearrange("p (g e) -> p g e", g=NG)
            for g in range(NG):
                nc.vector.tensor_single_scalar(oh[:, g, :], iotaE_sb[:],
                                               fields[:, g, 0:1], ALU.is_equal)
                nc.vector.tensor_single_scalar(st2[:, g, :], iotaE_sb[:],
                                               fields[:, g, 1:2], ALU.is_equal)
            nc.vector.tensor_tensor(st2[:, :, :], st2[:, :, :], oh[:], ALU.add)
            p_s = rps.tile([1, 2 * NG * E], dt.float32, tag="p_s")
            nc.tensor.matmul(p_s[:], ones_sb[:], statpk[:], start=True, stop=True)
            statrow = rp.tile([1, 2 * NG * E], dt.float32, tag="statrow")
            nc.vector.tensor_copy(statrow[:], p_s[:])

            # ship fields + stats, AllGather
            nc.sync.dma_start(
                agin1[:].rearrange("(g p f) -> p g f", p=128, f=4), fields[:])
            nc.sync.dma_start(agin2[:], statrow[0:1, :])
            if 'ag' in skip:
                nc.sync.dma_start(agout1[0:TS * 4], agin1[:])
                nc.sync.dma_start(agout2[0:64], agin2[:])
            else:
                nc.gpsimd.collective_compute(
                    "AllGather", ALU.bypass, replica_groups=rg,
                    ins=[agin1[:]], outs=[agout1[:]])
                nc.gpsimd.collective_compute(
                    "AllGather", ALU.bypass, replica_groups=rg,
                    ins=[agin2[:]], outs=[agout2[:]])

        if phases == 1:
            with tc.tile_pool(name="dbg1", bufs=1) as dp:
                dsb = dp.tile([128, 128], dt.float32, tag="dsb")
                nc.sync.dma_start(
                    dsb[:], agout1[:].rearrange("(p q) -> p q", p=128))
                nc.sync.dma_start(
                    out[0:16, :].rearrange("(a) d -> (a d)").rearrange(
                        "(p q) -> p q", p=128), dsb[:])
                auxs0 = dp.tile([1, 1], dt.float32, tag="auxs0")
                nc.scalar.memzero(auxs0[:])
                nc.sync.dma_start(aux[:, :], auxs0[:])
            nc.compile()
            return nc

        # =========== PHASE 2: aux loss + compaction ===========
        with tc.tile_pool(name="compact", bufs=1) as cp:
            # aux loss from gathered stats
            statsum = cp.tile([1, NCORES * 64], dt.float32, tag="statsum")
            nc.scalar.memzero(statsum[:])
            nc.sync.dma_start(statsum[:], agout2[:])
            stat64 = cp.tile([1, 64], dt.float32, tag="stat64")
            nc.vector.tensor_reduce(
                stat64[:], statsum[:].rearrange("p (r q) -> p q r", r=NCORES),
                mybir.AxisListType.X, ALU.add)
            stat16 = cp.tile([1, 16], dt.float32, tag="stat16")
            nc.vector.tensor_reduce(
                stat16[:], stat64[:].rearrange("p (s g e) -> p s e g", s=2, g=4),
                mybir.AxisListType.X, ALU.add)
            auxm = cp.tile([1, E], dt.float32, tag="auxm")
            nc.vector.tensor_tensor(auxm[:], stat16[:, 0:E], stat16[:, E:2 * E],
                                    ALU.mult)
            auxs = cp.tile([1, 1], dt.float32, tag="auxs")
            nc.vector.tensor_reduce(auxs[:], auxm[:], mybir.AxisListType.X, ALU.add)
            nc.vector.tensor_scalar_mul(auxs[:], auxs[:], float(E) / (T * T * 2))
            nc.sync.dma_start(aux[:, :], auxs[:])

            # routing table in wrap-16 layout [16, 256, 4]
            rtw = cp.tile([16, T // 16, 4], dt.float32, tag="rtw")
            nc.scalar.memzero(rtw[:])
            nc.sync.dma_start(
                rtw[:],
                agout1[:].rearrange("(r g c p f) -> p (r g c) f",
                                    r=NCORES, g=4, c=8, p=16))
            iw_sb = cp.tile([16, T // 16], dt.float32, tag="iw")
            nc.scalar.memzero(iw_sb[:])
            nc.sync.dma_start(iw_sb[:], iwrap1[:, :])
            me_sb = cp.tile([16, 1], dt.float32, tag="me")
            nc.scalar.memzero(me_sb[:])
            nc.sync.dma_start(me_sb[:], myexp[:, :])

            sel1 = cp.tile([16, T // 16], dt.float32, tag="sel1")
            nc.vector.tensor_single_scalar(sel1[:], rtw[:, :, 0], me_sb[:, 0:1],
                                           ALU.is_equal)
            sel2 = cp.tile([16, T // 16], dt.float32, tag="sel2")
            nc.vector.tensor_single_scalar(sel2[:], rtw[:, :, 1], me_sb[:, 0:1],
                                           ALU.is_equal)
            # gate g = sel1*w1 + sel2*w2 ; value vg = (g+1)*sel - 1
            gt = cp.tile([16, T // 16], dt.float32, tag="gt")
            nc.vector.tensor_tensor(gt[:], sel1[:], rtw[:, :, 2], ALU.mult)
            gt2 = cp.tile([16, T // 16], dt.float32, tag="gt2")
            nc.vector.tensor_tensor(gt2[:], sel2[:], rtw[:, :, 3], ALU.mult)
            nc.vector.tensor_tensor(gt[:], gt[:], gt2[:], ALU.add)
            sel = cp.tile([16, T // 16], dt.float32, tag="sel")
            nc.vector.tensor_tensor(sel[:], sel1[:], sel2[:], ALU.add)
            # v = sel * (t+1) - 1
            vv = cp.tile([16, T // 16], dt.float32, tag="vv")
            nc.vector.tensor_tensor(vv[:], sel[:], iw_sb[:], ALU.mult)
            nc.vector.tensor_scalar_add(vv[:], vv[:], -1.0)
            # vg = sel * (g+1) - 1
            vg = cp.tile([16, T // 16], dt.float32, tag="vg")
            nc.vector.tensor_scalar_add(gt[:], gt[:], 1.0)
            nc.vector.tensor_tensor(vg[:], sel[:], gt[:], ALU.mult)
            nc.vector.tensor_scalar_add(vg[:], vg[:], -1.0)

            idxf = cp.tile([16, CW], dt.float32, tag="idxf")
            nf1 = cp.tile([1, 1], dt.uint32, tag="nf1")
            gf = cp.tile([16, CW], dt.float32, tag="gf")
            nf2 = cp.tile([1, 1], dt.uint32, tag="nf2")
            if 'sg' in skip:
                nc.scalar.memzero(idxf[:])
                nc.scalar.memzero(gf[:])
                nc.vector.tensor_tensor(idxf[:], idxf[:], vv[:, 0:CW], ALU.add)
                nc.vector.tensor_tensor(gf[:], gf[:], vg[:, 0:CW], ALU.add)
            else:
                nc.gpsimd.sparse_gather(idxf[:], vv[:], num_found=nf1[:])
                nc.gpsimd.sparse_gather(gf[:], vg[:], num_found=nf2[:])

            # hardware sparse_gather leaves the output tail UNWRITTEN (the
            # sim pads -1) -- mask slots >= num_found via select so stale
            # garbage (even NaN) never reaches the index/gate path.
            nff = cp.tile([1, 1], dt.float32, tag="nff")
            nc.vector.tensor_copy(nff[:], nf1[:])
            for k in range(16):
                nc.sync.dma_start(nfdram[k:k + 1], nff[0:1, :])
            nf16 = cp.tile([16, 1], dt.float32, tag="nf16")
            nc.scalar.memzero(nf16[:])
            nc.sync.dma_start(nf16[:], nfdram[:].rearrange("(p q) -> p q", q=1))
            iwc = cp.tile([16, CW], dt.float32, tag="iwc")
            nc.scalar.memzero(iwc[:])
            nc.sync.dma_start(iwc[:], iwrapC[:, :])
            vmask = cp.tile([16, CW], dt.uint8, tag="vmask")
            nc.vector.tensor_single_scalar(vmask[:], iwc[:], nf16[:, 0:1], ALU.is_lt)
            trash = cp.tile([16, CW], dt.float32, tag="trash")
            nc.scalar.activation(trash[:], trash[:], AF.Copy,
                                 bias=float(TRASH), scale=0.0)
            idxc = cp.tile([16, CW], dt.float32, tag="idxc")
            nc.vector.select(idxc[:], vmask[:], idxf[:], trash[:])
            gzero = cp.tile([16, CW], dt.float32, tag="gzero")
            nc.scalar.memzero(gzero[:])
            gc = cp.tile([16, CW], dt.float32, tag="gc")
            nc.vector.select(gc[:], vmask[:], gf[:], gzero[:])
            idx16 = cp.tile([16, CW], dt.int16, tag="idx16")
            nc.vector.tensor_copy(idx16[:], idxc[:])

            # replicate idx to all 8 gpsimd cores (16-partition stripes)
            nc.sync.dma_start(idxdram[:].rearrange("(c p) -> p c", p=16), idx16[:])
            nc.sync.dma_start(gdram[:].rearrange("(c p) -> p c", p=16), gc[:])

        if phases == 2:
            with tc.tile_pool(name="dbg2", bufs=1) as dp:
                df = dp.tile([16, CW], dt.float32, tag="df")
                nc.vector.tensor_copy(df[:], idx16[:])
                nc.sync.dma_start(
                    out[0:1, 0:C].rearrange("a (c p) -> (a p) c", p=16), df[:])
                nc.sync.dma_start(
                    out[1:2, 0:C].rearrange("a (c p) -> (a p) c", p=16), gc[:])
            nc.compile()
            return nc

        with (
            tc.tile_pool(name="ffn", bufs=1) as fp,
            tc.tile_pool(name="hpool", bufs=2) as hp,
            tc.tile_pool(name="gath", bufs=2) as gp,
            tc.tile_pool(name="fpsum", bufs=2, space="PSUM") as fps,
            tc.tile_pool(name="tpsum", bufs=2, space="PSUM") as tps,
        ):
            idx128 = fp.tile([128, CW], dt.int16, tag="idx128")
            nc.vector.tensor_copy(idx128[:], idx128[:])
            idv = idxdram[:].rearrange("(c p) -> p c", p=16)
            for k in range(8):
                nc.sync.dma_start(idx128[ts(k, 16), :], idv)
            g128 = fp.tile([128, C // 128], dt.float32, tag="g128")
            nc.vector.tensor_copy(g128[:], g128[:])
            nc.sync.dma_start(g128[:], gdram[:].rearrange("(j p) -> p j", p=128))

            # =========== PHASE 3+4: gather / FFN / scatter, 3 chunks ===========
            for ci, (c0, cn) in enumerate(CHUNKS):
                hbf = hp.tile([128, F // 128, 512], dt.bfloat16, tag="hbf")
                ngrp = cn // 128
                # per-128-slot gathers on rotating SWDGE queues (separate
                # tiles so each queue keeps its own completion-sem lane);
                # cast + transpose into xgT [128, 8, cn] bf16
                xgT = gp.tile([128, D // 128, 512], dt.bfloat16, tag="xgT")
                for j in range(ngrp):
                    q = 0
                    q0 = c0 + 128 * j
                    xgj = gp.tile([128, D], dt.float32, tag=f"xgq{q}")
                    if 'gather' in skip:
                        nc.scalar.memzero(xgj[:])
                    else:
                        nc.gpsimd.dma_gather(
                            xgj[:].rearrange("p (a d) -> p a d", a=1), xp[:, :],
                            idx128[:, q0 // 16:(q0 + 128) // 16],
                            num_idxs=128, num_idxs_reg=128, elem_size=D,
                            queue_num=q)
                    xgb = gp.tile([128, D], dt.bfloat16, tag="xgb")
                    nc.scalar.copy(xgb[:], xgj[:])
                    for k in range(D // 128):
                        ptb = tps.tile([128, 128], dt.bfloat16, tag="ptb")
                        nc.tensor.transpose(ptb[:], xgb[:, ts(k, 128)], identb_sb[:])
                        nc.vector.tensor_copy(xgT[:, k, ts(j, 128)], ptb[:])

                # FFN1: hT[f*128+p, t] = gelu(sum_k w1[k, f*128+p] xgT[k, t] + b1)
                for f in range(F // 128):
                    ph = fps.tile([128, 512], dt.float32, tag="ph")
                    for k in range(D // 128):
                        nc.tensor.matmul(ph[:, 0:cn], w1_sb[:, k, ts(f, 128)],
                                         xgT[:, k, 0:cn],
                                         start=(k == 0), stop=(k == D // 128 - 1))
                    nc.scalar.activation(hbf[:, f, 0:cn], ph[:, 0:cn], AF.Gelu,
                                         bias=b1_sb[:, f:f + 1])

                # FFN2: y[t, d] = sum_f hT[f, t] w2[f, d]; +b2, *gate
                ysb = gp.tile([128, 4, D], dt.float32, tag="ysb")
                for j in range(ngrp):
                    for dtile in range(D // 512):
                        py = fps.tile([128, 512], dt.float32, tag="py")
                        for k in range(F // 128):
                            nc.tensor.matmul(
                                py[:], hbf[:, k, ts(j, 128)],
                                w2_sb[:, k, ts(dtile, 512)],
                                start=(k == 0), stop=(k == F // 128 - 1))
                        yv = ysb[:, j, ts(dtile, 512)]
                        nc.vector.tensor_tensor(yv, py[:], b2_sb[:, ts(dtile, 512)],
                                                ALU.add)
                        nc.vector.tensor_scalar_mul(
                            yv, yv, g128[:, c0 // 128 + j:c0 // 128 + j + 1])
                if 'scatter' in skip and ci == 0:
                    nc.sync.dma_start(acc[0:128, 0:40].bitcast(dt.int16),
                                      idx128[:, :])
                    nc.sync.dma_start(acc[128:256, 0:10], g128[:, :])
                if 'scatter' not in skip:
                    nc.gpsimd.dma_scatter_add(
                        acc[:, :], ysb[:, 0:ngrp, :],
                        idx128[:, c0 // 16:(c0 + cn) // 16],
                        num_idxs=cn, num_idxs_reg=cn, elem_size=D,
                        queue_num=0)

        # =========== PHASE 5: ReduceScatter + output ===========
        with tc.tile_pool(name="outp", bufs=1) as op:
            if phases >= 4:
                nc.gpsimd.collective_compute(
                    "ReduceScatter", ALU.add, replica_groups=rg,
                    ins=[acc[0:T, :]], outs=[rsout[:, :]])
                src_dram = rsout
            else:
                src_dram = None
            o_sb = op.tile([128, TS // 128, D], dt.float32, tag="o_sb")
            nc.vector.tensor_copy(o_sb[:], o_sb[:])
            if src_dram is not None:
                nc.sync.dma_start(
                    o_sb[:], src_dram[:, :].rearrange("(g p) d -> p g d", p=128))
            else:
                nc.sync.dma_start(
                    o_sb[:], acc[0:TS, :].rearrange("(g p) d -> p g d", p=128))
            nc.sync.dma_start(out[:, :].rearrange("(g p) d -> p g d", p=128), o_sb[:])

    nc.compile()
    return nc


def make_in_maps(inputs):
    x = np.ascontiguousarray(np.asarray(inputs["x"], dtype=np.float32))
    router_w = np.asarray(inputs["router_w"], dtype=np.float32)
    router_b = np.asarray(inputs["router_b"], dtype=np.float32)
    W1 = np.asarray(inputs["W1"], dtype=np.float32)
    b1 = np.asarray(inputs["b1"], dtype=np.float32)
    W2 = np.asarray(inputs["W2"], dtype=np.float32)
    b2 = np.asarray(inputs["b2"], dtype=np.float32)

    flat = x.reshape(T, D)
    xp = np.zeros((XPAD, D), np.float32)
    xp[:T] = flat
    identf = np.eye(128, dtype=np.float32)
    identb = np.eye(128, dtype=ml_dtypes.bfloat16)
    iotaE = np.tile(np.arange(E, dtype=np.float32), (128, 1))
    onesP = np.ones((128, 1), np.float32)
    # wrap-16: element (p, c) is token t = 16c + p; store t+1
    iw = (np.arange(T, dtype=np.float32) + 1.0).reshape(T // 16, 16).T.copy()
    iwc = np.arange(C, dtype=np.float32).reshape(CW, 16).T.copy()
    rbT = router_b.reshape(E, 1)

    in_maps = []
    for r in range(NCORES):
        in_maps.append({
            "xs": np.ascontiguousarray(flat[r * TS:(r + 1) * TS]),
            "xp": xp,
            "rw": router_w,
            "rbT": rbT,
            "w1": np.ascontiguousarray(W1[r]),
            "b1T": np.ascontiguousarray(b1[r].reshape(F // 128, 128).T),
            "w2": np.ascontiguousarray(W2[r]),
            "b2T": np.tile(b2[r], (128, 1)),
            "identf": identf,
            "identb": identb,
            "iotaE": iotaE,
            "onesP": onesP,
            "iwrap1": iw,
            "iwrapC": iwc,
            "myexp": np.full((16, 1), float(r), np.float32),
        })
    return in_maps


_NC = None


def kernel(**inputs):
    global _NC
    if _NC is None:
        _NC = build_nc()
    in_maps = make_in_maps(inputs)
    res = run_bass_kernel_spmd(_NC, in_maps, core_ids=list(range(NCORES)))
    out = np.concatenate([res.results[r]["out"] for r in range(NCORES)], axis=0)
    aux = np.float32(res.results[0]["aux"][0, 0])
    return out.reshape(np.asarray(inputs["x"]).shape), aux


if __name__ == "__main__":
    nc = build_nc()
    print("build + compile OK;",
          sum(1 for _ in nc.m.functions[0].basicblocks[0].instructions
              ) if hasattr(nc.m.functions[0], "basicblocks") else "")
